# revision 23
# baseline (speedup 1.0000x reference)
"""BernoulliRBF retrieval kernel for 8 trn2 NeuronCores.

Math: for each query n, over each reference set (pos/neg):
    score[n,m] = 2 xs_n.ys_m - |ys_m|^2 - |xs_n|^2 - wb
    log_count[n] = LSE_m score[n,m]
Outputs: log_p_x = log_pos - logaddexp(log_pos, log_neg), log_weight_count.

Device layout (per core; cores 0-3 hold pos shards, 4-7 neg shards,
M-sharded 8192 refs/core), TRANSPOSED vs the usual: references live on
the partition axis, queries on the free axis.

    psum[p=ref, n=query] = sum_k ys[k, ref] * 2xs[k, query]   (fp16 matmuls,
                              1 cyc/col + fast weight load; fp32 PSUM accum)
    u = exp(psum + bias_p)    ACT straight from PSUM (dodges the SBUF-source
                              errata), bias = -|ys|^2 - C per-partition (the
                              whole reason for the transposed layout: no
                              per-column DVE bias add is needed), bf16 out
    DVE merges each tree's ref-tiles' u into a root (bf16 adds, 2x mode)
    roots [128, 2048] DMA out; host reduces over partitions/roots/cores
    in float64: log_count = C + log(sum) - |xs|^2 - wb.

C is a single per-set shift from a host-side subsample (safe: it only
needs to be within ~±80 of the true max; exp under/overflow margins are
huge in fp32/bf16).
"""
import os
import numpy as np
from contextlib import ExitStack

N, M, D = 2048, 32768, 256
NCORES = 8
CORES_PER_SET = 4
SHARD = M // CORES_PER_SET      # 8192 refs per core
NRT = SHARD // 128              # 64 ref tiles per core
TREE_SIZES = [4] * 14 + [2] * 3  # ref tiles per accumulation tree; short
RAW_TILES = 2                    # trees at the end shrink the kernel tail;
NTREE = len(TREE_SIZES) + RAW_TILES  # the last tiles ship unmerged
C_MARGIN = 12.0
SAMPLE_STRIDE = 64              # 512-point subsample for the C shift

LAST_EXEC_NS = None             # set when BASS_TRACE=1

_cache = {}


def _build():
    import concourse.tile as tile
    from concourse import bacc, mybir

    F32, F16, BF16 = mybir.dt.float32, mybir.dt.float16, mybir.dt.bfloat16

    nc = bacc.Bacc("TRN2", target_bir_lowering=False, debug=False)
    # moving operand: queries, [k-half, k, n]
    A = nc.dram_tensor("A", [2, 128, N], F16, kind="ExternalInput").ap()
    # stationary operand: refs, [k-half, k, m]
    B = nc.dram_tensor("B", [2, 128, SHARD], F16, kind="ExternalInput").ap()
    # per-ref bias (-|ys|^2 - C), [p, ref-tile]
    WB = nc.dram_tensor("WB", [128, NRT], F32, kind="ExternalInput").ap()
    # tree roots out
    U = nc.dram_tensor("U", [128, NTREE, N], BF16, kind="ExternalOutput").ap()

    with tile.TileContext(nc) as tc:
        with ExitStack() as ctx:
            sing = ctx.enter_context(tc.tile_pool(name="sing", bufs=1))
            psums = ctx.enter_context(tc.tile_pool(name="psum", bufs=2, space="PSUM"))
            upool = ctx.enter_context(tc.tile_pool(name="u", bufs=8))

            wb_sb = sing.tile([128, NRT], F32)
            nc.sync.dma_start(out=wb_sb[:], in_=WB)
            a_sb = sing.tile([128, 2, N], F16)
            b_sb = sing.tile([128, 2, SHARD], F16)
            # Startup-critical path: tile 0 needs b[0:128] (both halves)
            # and ALL of a. Small leading pieces, a split across both
            # queues, b chunks graded bigger toward the tail.
            # Startup wave: per-queue DMA bandwidth is only ~100 GB/s, so
            # the critical tile-0 inputs (b[0:256] + the a query-halves
            # in consumption order) are spread across the three
            # DMA-capable queues (sync/gpsimd/scalar); scalar is idle
            # this early.
            nc.sync.dma_start(out=b_sb[:, 0, 0:256], in_=B[0][:, 0:256])
            nc.sync.dma_start(out=b_sb[:, 1, 0:256], in_=B[1][:, 0:256])
            nc.gpsimd.dma_start(out=a_sb[:, 0, 0:1024], in_=A[0][:, 0:1024])
            nc.scalar.dma_start(out=a_sb[:, 1, 0:1024], in_=A[1][:, 0:1024])
            nc.sync.dma_start(out=a_sb[:, 0, 1024:], in_=A[0][:, 1024:])
            nc.gpsimd.dma_start(out=a_sb[:, 1, 1024:], in_=A[1][:, 1024:])
            # tiles 1-7 are consumed before the regular chunk stream
            # catches up — ship their refs in the startup wave too
            nc.scalar.dma_start(out=b_sb[:, 0, 256:1024], in_=B[0][:, 256:1024])
            nc.scalar.dma_start(out=b_sb[:, 1, 256:1024], in_=B[1][:, 256:1024])
            b_chunks = [1024, 2048, 3072, 4096, 5120, 6144, 7168, SHARD]
            for mc in range(len(b_chunks) - 1):
                sl = slice(b_chunks[mc], b_chunks[mc + 1])
                eng = nc.gpsimd if mc % 2 == 0 else nc.sync
                for h in range(2):
                    eng.dma_start(out=b_sb[:, h, sl], in_=B[h][:, sl])

            # PE warmup: dummy matmuls on zeroed tiles while input DMAs
            # land, so the HAM clock gate opens (1.2 -> 2.4 GHz) before
            # the first real tile. No data dependency on any DMA.
            warm_w = sing.tile([128, 128], F16)
            warm_a = sing.tile([128, 512], F16)
            nc.vector.memset(warm_w[:], 0.0)
            nc.vector.memset(warm_a[:], 0.0)
            psum = psums.tile([128, N], F32)
            for _ in range(10):
                nc.tensor.matmul(
                    psum[:, 0:512], warm_w[:], warm_a[:], start=True, stop=True
                )

            def emit_tile(r):
                """MMs + exp for ref-tile r; returns its u tile (bf16)."""
                psum = psums.tile([128, N], F32)
                b_slices = [
                    b_sb[:, h, r * 128:(r + 1) * 128] for h in range(2)
                ]
                u = upool.tile([128, N], BF16)
                if r < 4:
                    # ramp phase: finish query-halves and exp them as
                    # soon as their a-columns have landed, so the ACT
                    # chain (the kernel's wall) starts ~8us earlier
                    for half in range(2):
                        hsl = slice(half * 1024, (half + 1) * 1024)
                        for h in range(2):
                            for c in (2 * half, 2 * half + 1):
                                nc.tensor.matmul(
                                    psum[:, c * 512:(c + 1) * 512],
                                    b_slices[h],
                                    a_sb[:, h, c * 512:(c + 1) * 512],
                                    start=(h == 0),
                                    stop=(h == 1),
                                )
                        nc.scalar.activation(
                            out=u[:, hsl],
                            in_=psum[:, hsl],
                            func=mybir.ActivationFunctionType.Exp,
                            bias=wb_sb[:, r:r + 1],
                            scale=1.0,
                        )
                else:
                    for h in range(2):
                        for c in range(4):
                            nc.tensor.matmul(
                                psum[:, c * 512:(c + 1) * 512],
                                b_slices[h],
                                a_sb[:, h, c * 512:(c + 1) * 512],
                                start=(h == 0),
                                stop=(h == 1),
                            )
                    nc.scalar.activation(
                        out=u[:],
                        in_=psum[:],
                        func=mybir.ActivationFunctionType.Exp,
                        bias=wb_sb[:, r:r + 1],
                        scale=1.0,
                    )
                return u

            r = 0
            for tr, tpt in enumerate(TREE_SIZES):
                part0 = None     # merged pair (tiles 0+1 of this tree)
                prev = None      # unmerged odd tile
                for i in range(tpt):
                    u = emit_tile(r)
                    r += 1
                    if i % 2 == 0:
                        prev = u
                        continue
                    merged = upool.tile([128, N], BF16)
                    nc.vector.tensor_add(merged[:], prev[:], u[:])
                    if i == 1 and tpt == 4:
                        part0 = merged
                        continue
                    if tpt == 4:
                        root = upool.tile([128, N], BF16)
                        nc.vector.tensor_add(root[:], part0[:], merged[:])
                    else:
                        root = merged
                    eng = nc.sync if tr % 2 == 0 else nc.gpsimd
                    eng.dma_start(out=U[:, tr, :], in_=root[:])
            # tail tiles ship unmerged: nothing between the last exp and
            # its store; the final store is split across both queues
            for j in range(RAW_TILES):
                u = emit_tile(r)
                tr = len(TREE_SIZES) + j
                if j == RAW_TILES - 1:
                    nc.sync.dma_start(out=U[:, tr, 0:N // 2], in_=u[:, 0:N // 2])
                    nc.gpsimd.dma_start(out=U[:, tr, N // 2:], in_=u[:, N // 2:])
                else:
                    nc.sync.dma_start(out=U[:, tr, :], in_=u[:])
                r += 1

    nc.compile()
    return nc


def _prep_set(x, data, scale):
    """Host-side prep for one reference set."""
    xs = (x * scale[None, :]).astype(np.float32)          # match reference rounding
    ys = (data * scale[None, :]).astype(np.float32)
    A = np.ascontiguousarray((2.0 * xs).T).reshape(2, 128, N).astype(np.float16)
    BT = np.ascontiguousarray(ys.T).reshape(2, 128, M).astype(np.float16)
    w = -((ys.astype(np.float64) ** 2).sum(axis=1))       # [M], float64
    # single per-set shift from a subsample
    samp = ys[::SAMPLE_STRIDE]
    t_s = 2.0 * (xs @ samp.T) + w[::SAMPLE_STRIDE][None, :].astype(np.float32)
    C = float(t_s.max()) + C_MARGIN
    xsq = (xs.astype(np.float64) ** 2).sum(axis=1)        # [N], float64
    return A, BT, w, C, xsq


def kernel(x, data_pos, data_neg, scales_pos, scales_neg, weight_bias):
    global LAST_EXEC_NS
    import ml_dtypes
    from concourse.bass_utils import run_bass_kernel_spmd

    x = np.asarray(x, dtype=np.float32)
    data_pos = np.asarray(data_pos, dtype=np.float32)
    data_neg = np.asarray(data_neg, dtype=np.float32)
    scales_pos = np.asarray(scales_pos, dtype=np.float32)
    scales_neg = np.asarray(scales_neg, dtype=np.float32)
    weight_bias = np.asarray(weight_bias, dtype=np.float32)

    if "nc" not in _cache:
        _cache["nc"] = _build()
    nc = _cache["nc"]

    prep_p = _prep_set(x, data_pos, scales_pos)
    prep_n = _prep_set(x, data_neg, scales_neg)

    in_maps = []
    for core in range(NCORES):
        A_, BT_, w_, C_, _ = prep_p if core < CORES_PER_SET else prep_n
        sh = core % CORES_PER_SET
        sl = slice(sh * SHARD, (sh + 1) * SHARD)
        wb = (w_[sl] - C_).astype(np.float32).reshape(NRT, 128).T
        in_maps.append(
            {
                "A": A_,
                "B": np.ascontiguousarray(BT_[:, :, sl]),
                "WB": np.ascontiguousarray(wb),
            }
        )

    trace = os.environ.get("BASS_TRACE", "") not in ("", "0")
    try:
        res = run_bass_kernel_spmd(nc, in_maps, list(range(NCORES)), trace=trace)
    except ModuleNotFoundError:
        # profiling hook unavailable in this environment — run untraced
        res = run_bass_kernel_spmd(nc, in_maps, list(range(NCORES)), trace=False)
    LAST_EXEC_NS = res.exec_time_ns

    # host combine in float64
    def reduce_set(cores, C, xsq, wb):
        tot = np.zeros(N)
        for core in cores:
            u = res.results[core]["U"]                    # [128, NTREE, N] bf16
            tot += u.astype(np.float64).sum(axis=(0, 1))
        return C + np.log(tot) - xsq - float(wb)

    log_pos = reduce_set(range(CORES_PER_SET), prep_p[3], prep_p[4], weight_bias[0])
    log_neg = reduce_set(
        range(CORES_PER_SET, NCORES), prep_n[3], prep_n[4], weight_bias[1]
    )
    log_weight = np.logaddexp(log_pos, log_neg)
    log_p_x = log_pos - log_weight
    return (log_p_x.astype(np.float32), log_weight.astype(np.float32))


# revision 24
# speedup vs baseline: 1.0108x; 1.0108x over previous
"""BernoulliRBF retrieval kernel for 8 trn2 NeuronCores.

Math: for each query n, over each reference set (pos/neg):
    score[n,m] = 2 xs_n.ys_m - |ys_m|^2 - |xs_n|^2 - wb
    log_count[n] = LSE_m score[n,m]
Outputs: log_p_x = log_pos - logaddexp(log_pos, log_neg), log_weight_count.

Device layout (per core; cores 0-3 hold pos shards, 4-7 neg shards,
M-sharded 8192 refs/core), TRANSPOSED vs the usual: references live on
the partition axis, queries on the free axis.

    psum[p=ref, n=query] = sum_k ys[k, ref] * 2xs[k, query]   (fp16 matmuls,
                              1 cyc/col + fast weight load; fp32 PSUM accum)
    u = exp(psum + bias_p)    ACT straight from PSUM (dodges the SBUF-source
                              errata), bias = -|ys|^2 - C per-partition (the
                              whole reason for the transposed layout: no
                              per-column DVE bias add is needed), bf16 out
    DVE merges each tree's ref-tiles' u into a root (bf16 adds, 2x mode)
    roots [128, 2048] DMA out; host reduces over partitions/roots/cores
    in float64: log_count = C + log(sum) - |xs|^2 - wb.

C is a single per-set shift from a host-side subsample (safe: it only
needs to be within ~±80 of the true max; exp under/overflow margins are
huge in fp32/bf16).
"""
import os
import numpy as np
from contextlib import ExitStack

N, M, D = 2048, 32768, 256
NCORES = 8
CORES_PER_SET = 4
SHARD = M // CORES_PER_SET      # 8192 refs per core
NRT = SHARD // 128              # 64 ref tiles per core
TREE_SIZES = [4] * 14 + [2] * 3  # ref tiles per accumulation tree; short
RAW_TILES = 2                    # trees at the end shrink the kernel tail;
NTREE = len(TREE_SIZES) + RAW_TILES  # the last tiles ship unmerged
C_MARGIN = 12.0
SAMPLE_STRIDE = 64              # 512-point subsample for the C shift

LAST_EXEC_NS = None             # set when BASS_TRACE=1

_cache = {}


def _build():
    import concourse.tile as tile
    from concourse import bacc, mybir

    F32, F16, BF16 = mybir.dt.float32, mybir.dt.float16, mybir.dt.bfloat16

    nc = bacc.Bacc("TRN2", target_bir_lowering=False, debug=False)
    # moving operand: queries, [k-half, k, n]
    A = nc.dram_tensor("A", [2, 128, N], F16, kind="ExternalInput").ap()
    # stationary operand: refs, [k-half, k, m]
    B = nc.dram_tensor("B", [2, 128, SHARD], F16, kind="ExternalInput").ap()
    # per-ref bias (-|ys|^2 - C), [p, ref-tile]
    WB = nc.dram_tensor("WB", [128, NRT], F32, kind="ExternalInput").ap()
    # tree roots out
    U = nc.dram_tensor("U", [128, NTREE, N], BF16, kind="ExternalOutput").ap()

    with tile.TileContext(nc) as tc:
        with ExitStack() as ctx:
            sing = ctx.enter_context(tc.tile_pool(name="sing", bufs=1))
            psums = ctx.enter_context(tc.tile_pool(name="psum", bufs=2, space="PSUM"))
            upool = ctx.enter_context(tc.tile_pool(name="u", bufs=8))

            wb_sb = sing.tile([128, NRT], F32)
            nc.sync.dma_start(out=wb_sb[:], in_=WB)
            a_sb = sing.tile([128, 2, N], F16)
            b_sb = sing.tile([128, 2, SHARD], F16)
            # Startup-critical path: tile 0 needs b[0:128] (both halves)
            # and ALL of a. Small leading pieces, a split across both
            # queues, b chunks graded bigger toward the tail.
            # Startup wave: per-queue DMA bandwidth is only ~100 GB/s, so
            # the critical tile-0 inputs (b[0:256] + the a query-halves
            # in consumption order) are spread across the three
            # DMA-capable queues (sync/gpsimd/scalar); scalar is idle
            # this early.
            nc.sync.dma_start(out=b_sb[:, 0, 0:256], in_=B[0][:, 0:256])
            nc.sync.dma_start(out=b_sb[:, 1, 0:256], in_=B[1][:, 0:256])
            nc.gpsimd.dma_start(out=a_sb[:, 0, 0:1024], in_=A[0][:, 0:1024])
            nc.scalar.dma_start(out=a_sb[:, 1, 0:1024], in_=A[1][:, 0:1024])
            nc.sync.dma_start(out=a_sb[:, 0, 1024:], in_=A[0][:, 1024:])
            nc.gpsimd.dma_start(out=a_sb[:, 1, 1024:], in_=A[1][:, 1024:])
            # tiles 1-7 are consumed before the regular chunk stream
            # catches up — ship their refs in the startup wave too
            nc.scalar.dma_start(out=b_sb[:, 0, 256:1024], in_=B[0][:, 256:1024])
            nc.scalar.dma_start(out=b_sb[:, 1, 256:1024], in_=B[1][:, 256:1024])
            b_chunks = [1024, 2048, 3072, 4096, 5120, 6144, 7168, SHARD]
            for mc in range(len(b_chunks) - 1):
                sl = slice(b_chunks[mc], b_chunks[mc + 1])
                eng = nc.gpsimd if mc % 2 == 0 else nc.sync
                for h in range(2):
                    eng.dma_start(out=b_sb[:, h, sl], in_=B[h][:, sl])

            # PE warmup: dummy matmuls on zeroed tiles while input DMAs
            # land, so the HAM clock gate opens (1.2 -> 2.4 GHz) before
            # the first real tile. No data dependency on any DMA.
            warm_w = sing.tile([128, 128], F16)
            warm_a = sing.tile([128, 512], F16)
            nc.vector.memset(warm_w[:], 0.0)
            nc.vector.memset(warm_a[:], 0.0)
            psum = psums.tile([128, N], F32)
            for _ in range(10):
                nc.tensor.matmul(
                    psum[:, 0:512], warm_w[:], warm_a[:], start=True, stop=True
                )

            def emit_tile(r):
                """MMs + exp for ref-tile r; returns its u tile (bf16)."""
                psum = psums.tile([128, N], F32)
                b_slices = [
                    b_sb[:, h, r * 128:(r + 1) * 128] for h in range(2)
                ]
                u = upool.tile([128, N], BF16)
                if r < 2:
                    # ramp phase: finish query-halves and exp them as
                    # soon as their a-columns have landed, so the ACT
                    # chain (the kernel's wall) starts ~8us earlier
                    for half in range(2):
                        hsl = slice(half * 1024, (half + 1) * 1024)
                        for h in range(2):
                            for c in (2 * half, 2 * half + 1):
                                nc.tensor.matmul(
                                    psum[:, c * 512:(c + 1) * 512],
                                    b_slices[h],
                                    a_sb[:, h, c * 512:(c + 1) * 512],
                                    start=(h == 0),
                                    stop=(h == 1),
                                )
                        nc.scalar.activation(
                            out=u[:, hsl],
                            in_=psum[:, hsl],
                            func=mybir.ActivationFunctionType.Exp,
                            bias=wb_sb[:, r:r + 1],
                            scale=1.0,
                        )
                else:
                    for h in range(2):
                        for c in range(4):
                            nc.tensor.matmul(
                                psum[:, c * 512:(c + 1) * 512],
                                b_slices[h],
                                a_sb[:, h, c * 512:(c + 1) * 512],
                                start=(h == 0),
                                stop=(h == 1),
                            )
                    nc.scalar.activation(
                        out=u[:],
                        in_=psum[:],
                        func=mybir.ActivationFunctionType.Exp,
                        bias=wb_sb[:, r:r + 1],
                        scale=1.0,
                    )
                return u

            r = 0
            for tr, tpt in enumerate(TREE_SIZES):
                part0 = None     # merged pair (tiles 0+1 of this tree)
                prev = None      # unmerged odd tile
                for i in range(tpt):
                    u = emit_tile(r)
                    r += 1
                    if i % 2 == 0:
                        prev = u
                        continue
                    merged = upool.tile([128, N], BF16)
                    nc.vector.tensor_add(merged[:], prev[:], u[:])
                    if i == 1 and tpt == 4:
                        part0 = merged
                        continue
                    if tpt == 4:
                        root = upool.tile([128, N], BF16)
                        nc.vector.tensor_add(root[:], part0[:], merged[:])
                    else:
                        root = merged
                    eng = nc.sync if tr % 2 == 0 else nc.gpsimd
                    eng.dma_start(out=U[:, tr, :], in_=root[:])
            # tail tiles ship unmerged: nothing between the last exp and
            # its store; the final store is split across both queues
            for j in range(RAW_TILES):
                u = emit_tile(r)
                tr = len(TREE_SIZES) + j
                if j == RAW_TILES - 1:
                    nc.sync.dma_start(out=U[:, tr, 0:N // 2], in_=u[:, 0:N // 2])
                    nc.gpsimd.dma_start(out=U[:, tr, N // 2:], in_=u[:, N // 2:])
                else:
                    nc.sync.dma_start(out=U[:, tr, :], in_=u[:])
                r += 1

    nc.compile()
    return nc


def _prep_set(x, data, scale):
    """Host-side prep for one reference set."""
    xs = (x * scale[None, :]).astype(np.float32)          # match reference rounding
    ys = (data * scale[None, :]).astype(np.float32)
    A = np.ascontiguousarray((2.0 * xs).T).reshape(2, 128, N).astype(np.float16)
    BT = np.ascontiguousarray(ys.T).reshape(2, 128, M).astype(np.float16)
    w = -((ys.astype(np.float64) ** 2).sum(axis=1))       # [M], float64
    # single per-set shift from a subsample
    samp = ys[::SAMPLE_STRIDE]
    t_s = 2.0 * (xs @ samp.T) + w[::SAMPLE_STRIDE][None, :].astype(np.float32)
    C = float(t_s.max()) + C_MARGIN
    xsq = (xs.astype(np.float64) ** 2).sum(axis=1)        # [N], float64
    return A, BT, w, C, xsq


def kernel(x, data_pos, data_neg, scales_pos, scales_neg, weight_bias):
    global LAST_EXEC_NS
    import ml_dtypes
    from concourse.bass_utils import run_bass_kernel_spmd

    x = np.asarray(x, dtype=np.float32)
    data_pos = np.asarray(data_pos, dtype=np.float32)
    data_neg = np.asarray(data_neg, dtype=np.float32)
    scales_pos = np.asarray(scales_pos, dtype=np.float32)
    scales_neg = np.asarray(scales_neg, dtype=np.float32)
    weight_bias = np.asarray(weight_bias, dtype=np.float32)

    if "nc" not in _cache:
        _cache["nc"] = _build()
    nc = _cache["nc"]

    prep_p = _prep_set(x, data_pos, scales_pos)
    prep_n = _prep_set(x, data_neg, scales_neg)

    in_maps = []
    for core in range(NCORES):
        A_, BT_, w_, C_, _ = prep_p if core < CORES_PER_SET else prep_n
        sh = core % CORES_PER_SET
        sl = slice(sh * SHARD, (sh + 1) * SHARD)
        wb = (w_[sl] - C_).astype(np.float32).reshape(NRT, 128).T
        in_maps.append(
            {
                "A": A_,
                "B": np.ascontiguousarray(BT_[:, :, sl]),
                "WB": np.ascontiguousarray(wb),
            }
        )

    trace = os.environ.get("BASS_TRACE", "") not in ("", "0")
    try:
        res = run_bass_kernel_spmd(nc, in_maps, list(range(NCORES)), trace=trace)
    except ModuleNotFoundError:
        # profiling hook unavailable in this environment — run untraced
        res = run_bass_kernel_spmd(nc, in_maps, list(range(NCORES)), trace=False)
    LAST_EXEC_NS = res.exec_time_ns

    # host combine in float64
    def reduce_set(cores, C, xsq, wb):
        tot = np.zeros(N)
        for core in cores:
            u = res.results[core]["U"]                    # [128, NTREE, N] bf16
            tot += u.astype(np.float64).sum(axis=(0, 1))
        return C + np.log(tot) - xsq - float(wb)

    log_pos = reduce_set(range(CORES_PER_SET), prep_p[3], prep_p[4], weight_bias[0])
    log_neg = reduce_set(
        range(CORES_PER_SET, NCORES), prep_n[3], prep_n[4], weight_bias[1]
    )
    log_weight = np.logaddexp(log_pos, log_neg)
    log_p_x = log_pos - log_weight
    return (log_p_x.astype(np.float32), log_weight.astype(np.float32))
